# revision 74
# baseline (speedup 1.0000x reference)
"""Trainium2 Bass kernel for nn_AdaptiveNet_SLSTM (2-layer SLSTM + FC).

Sharding: data-parallel over the inner batch dim (dim 1, size 1024) across
8 NeuronCores -> 128 rows/core; the sequential scan over dim 0 (128 steps)
runs locally per core with zero inter-core communication. Spike delta-
modulation encoding and weight layout prep run on host (0.01% of FLOPs).

Per-core design (v10 — recurrence chain latency minimized; TimelineSim
327.1us vs 408.1us baseline):
- States kept transposed [H=128 part, T_loc=128 free] so elementwise outputs
  feed the next step's matmuls directly as the moving operand.
- All 4 gates of a layer accumulate in one PSUM bank [128, 4*128]; biases are
  folded into matmuls (ones-row on the spike input for layer 1, rank-1
  matmuls for layer 2). All slack matmuls are emitted as 32-column
  granules (~13ns each) so a later-arriving chain matmul is blocked on
  the in-order PE for at most one granule.
- Per layer per step: one sigmoid over [i,f,g] (gates the elementwise front)
  and a separate o-gate sigmoid (needed only by the tail ~1.3us later, runs
  in an ACT idle window).
- Half-cell state hc = c/2 kept in bf16:  hc' = sig(f)*hc + (sig(2 z_g)-0.5)
  *sig(i), tanh(c') = tanh(2*hc') via the activation input scale. Front/tail
  elementwise ops are all-bf16 TensorTensor (2x DVE mode) except one
  scalar_tensor_tensor per layer.
- The membrane never materializes: mem = h - thr*spk_prev is folded into the
  matmuls (Whh@mem = Whh_h@h - Whh_s@spk_prev, and likewise for the FC
  reduction), so the per-step recurrence chain is
      h -> Whh_h matmuls -> sigmoid -> front -> tanh -> h'
  with every other matmul operand (spikes, bias, W*x) carrying >= 1
  iteration of slack. The spike compare spk = (h > RP) and the running
  threshold RP = thr*spk + thr run on DVE with lowest scheduler priority
  (the Pool engine only supports integer ALU ops), off the critical path.
- Matmuls into the same PSUM tile execute in emission order (range-based
  WAW serialization), so each z2 bank's matmuls are emitted together in
  operand-arrival order: bias, Whh_s@spk2, W2ih@spk1, Whh_h@h2 last; the
  group is prefetched one iteration ahead of its sigmoid. z1's slack
  matmuls (W1ih @ spikes, Whh_s) prefetch one step ahead likewise, so at
  the recurrence boundary only 3 Whh_h matmuls precede the sigmoid: the
  o-gate Whh_h matmul is deferred past the ifg-sigmoid emission (it only
  gates the slack o-sigmoid), and the fc matmuls are emitted last — slack
  PE work never sits ahead of chain matmuls in the scheduler's
  priority-ordered ready queue.
- The two layers are software-pipelined with a 2-iteration lag (iteration
  `it` emits layer 1 of step `it` and layer 2 of step `it-2`), which gives
  every layer-2 dependency >= 1 full iteration of slack; both layers'
  recurrence loops then bind dep-limited at the same ~2.46us steady-state
  period with ACT ~89% occupied.
- The output reduction sum_b(fcW @ mem2_b) accumulates in a PSUM bank via
  two tiny N=128 matmuls per step (fcW@h2 - thr*fcW@spk2_prev); final scale
  1/128 + fc bias in one tensor_scalar op.
- Matmuls bf16 (weights + bounded states; spikes are exact in bf16), PSUM
  fp32.
"""

import os
import sys

sys.path.insert(0, "/opt/trn_rl_repo")

import numpy as np
import ml_dtypes

import concourse.bass as bass
import concourse.bacc as bacc
import concourse.mybir as mybir
from concourse.tile import TileContext
from concourse.bass_utils import run_bass_kernel_spmd

BF16 = ml_dtypes.bfloat16
H = 128          # hidden size
B = 128          # scan steps (x dim 0)
T = 1024         # inner batch (x dim 1)
NCORES = 8
TLOC = T // NCORES  # 128 rows per core
F3 = 42          # 14 features x 3 thresholds
KIN = F3 + 1     # + ones row for layer-1 bias
NCLS = 8
THRESHOLDS = np.array([9.9893e-06, 2.9968e-05, 5.9936e-05], dtype=np.float32)
QORDER = [0, 1, 2, 3]  # gate order kept as PyTorch (i, f, g, o)
SPK_CHUNKS = 32
SPC = B // SPK_CHUNKS  # steps per chunk

LAST_RESULT = None  # BassKernelResults of the most recent run (for test.py)


def _build(thr1: float, thr2: float, reps: int = 1):
    nc = bacc.Bacc(None, target_bir_lowering=False)
    f32 = mybir.dt.float32
    bf16 = mybir.dt.bfloat16
    ACT = mybir.ActivationFunctionType
    OP = mybir.AluOpType

    spk_d = nc.dram_tensor("spk", [KIN, B * TLOC], bf16, kind="ExternalInput")
    w1ih_d = nc.dram_tensor("w1ih", [KIN, 4 * H], bf16, kind="ExternalInput")
    w1hh_d = nc.dram_tensor("w1hh", [H, 4 * H], bf16, kind="ExternalInput")
    w1hs_d = nc.dram_tensor("w1hs", [H, 4 * H], bf16, kind="ExternalInput")
    w2ih_d = nc.dram_tensor("w2ih", [H, 4 * H], bf16, kind="ExternalInput")
    w2hh_d = nc.dram_tensor("w2hh", [H, 4 * H], bf16, kind="ExternalInput")
    w2hs_d = nc.dram_tensor("w2hs", [H, 4 * H], bf16, kind="ExternalInput")
    b2l_d = nc.dram_tensor("b2l", [1, 4 * H], bf16, kind="ExternalInput")
    ind_d = nc.dram_tensor("ind", [1, 4 * H], bf16, kind="ExternalInput")
    fcwh_d = nc.dram_tensor("fcwh", [H, NCLS], bf16, kind="ExternalInput")
    fcws_d = nc.dram_tensor("fcws", [H, NCLS], bf16, kind="ExternalInput")
    fcb_d = nc.dram_tensor("fcb", [NCLS, 1], f32, kind="ExternalInput")
    out_d = nc.dram_tensor("out", [NCLS, TLOC], f32, kind="ExternalOutput")

    with TileContext(nc) as tc:
        with (
            tc.tile_pool(name="consts", bufs=1) as cpool,
            tc.tile_pool(name="spk", bufs=1) as spool,
            tc.tile_pool(name="state", bufs=1) as stpool,
            tc.tile_pool(name="sig", bufs=1) as sigpool,
            tc.tile_pool(name="ew", bufs=1) as ewpool,
            tc.tile_pool(name="zp", bufs=1, space="PSUM") as zpool,
            tc.tile_pool(name="fcp", bufs=1, space="PSUM") as fcpool,
        ):
            w1ih = cpool.tile([KIN, 4 * H], bf16, tag="w1ih")
            nc.sync.dma_start(w1ih[:], w1ih_d[:])
            w1hh = cpool.tile([H, 4 * H], bf16, tag="w1hh")
            nc.gpsimd.dma_start(w1hh[:], w1hh_d[:])
            spk_t = []
            for c in range(SPK_CHUNKS):
                t = spool.tile([KIN, SPC * TLOC], bf16, tag=f"spk{c}")
                spk_t.append(t)
            # startup DMAs split across queues (spk0 via ACT, early
            # weights via the idle Pool queue) so the DGE pipelines gating
            # the first iterations run in parallel instead of serializing
            # on SP; only DMAs for later-needed tensors stay on SP
            nc.scalar.dma_start(spk_t[0][:], spk_d[:, 0:SPC * TLOC])
            w1hs = cpool.tile([H, 4 * H], bf16, tag="w1hs")
            nc.gpsimd.dma_start(w1hs[:], w1hs_d[:])
            w2ih = cpool.tile([H, 4 * H], bf16, tag="w2ih")
            nc.gpsimd.dma_start(w2ih[:], w2ih_d[:])
            w2hh = cpool.tile([H, 4 * H], bf16, tag="w2hh")
            nc.gpsimd.dma_start(w2hh[:], w2hh_d[:])
            w2hs = cpool.tile([H, 4 * H], bf16, tag="w2hs")
            nc.sync.dma_start(w2hs[:], w2hs_d[:])
            b2l = cpool.tile([1, 4 * H], bf16, tag="b2l")
            nc.sync.dma_start(b2l[:], b2l_d[:])
            ind = cpool.tile([1, 4 * H], bf16, tag="ind")
            nc.sync.dma_start(ind[:], ind_d[:])
            fcwh = cpool.tile([H, NCLS], bf16, tag="fcwh")
            nc.sync.dma_start(fcwh[:], fcwh_d[:])
            fcws = cpool.tile([H, NCLS], bf16, tag="fcws")
            nc.sync.dma_start(fcws[:], fcws_d[:])
            fcb = cpool.tile([NCLS, 1], f32, tag="fcb")
            nc.sync.dma_start(fcb[:], fcb_d[:])
            for c in range(1, SPK_CHUNKS):
                nc.sync.dma_start(spk_t[c][:],
                                  spk_d[:, c * SPC * TLOC:(c + 1) * SPC * TLOC])

            # All state/temp tiles are FIXED allocations (no pool rotation):
            # same-engine WAR/WAW is free via program order, cross-engine
            # waits go through the minimizer and consolidate to <=1 per inst.
            HC = stpool.tile([H, 2 * TLOC], bf16, tag="HC")
            hc1 = HC[:, 0:TLOC]
            hc2 = HC[:, TLOC:2 * TLOC]
            THC = stpool.tile([H, 2 * TLOC], bf16, tag="THC")
            # h state: double-buffered (read by PE matmuls one iter later)
            H1t = [stpool.tile([H, TLOC], bf16, tag=f"h1_{i}", name=f"h1_{i}")
                   for i in range(2)]
            H2t = [stpool.tile([H, TLOC], bf16, tag=f"h2_{i}", name=f"h2_{i}")
                   for i in range(2)]
            # spikes: triple-buffered — spk(b) written at iter b while
            # spk(b-1) feeds layer-2 input and spk(b-2) feeds the Whh_s
            # recurrent matmul
            spk1 = [stpool.tile([H, TLOC], bf16, tag=f"spk1_{i}",
                                name=f"spk1_{i}") for i in range(3)]
            spk2 = [stpool.tile([H, TLOC], bf16, tag=f"spk2_{i}",
                                name=f"spk2_{i}") for i in range(3)]
            RP1 = stpool.tile([H, TLOC], bf16, tag="RP1")
            RP2 = stpool.tile([H, TLOC], bf16, tag="RP2")
            nc.vector.memset(HC[:], 0.0)
            for tl in (H1t[1], H2t[1], spk1[1], spk1[2], spk2[1], spk2[2]):
                nc.vector.memset(tl[:], 0.0)
            nc.gpsimd.memset(RP1[:], thr1)
            nc.gpsimd.memset(RP2[:], thr2)
            S1t = [sigpool.tile([H, 4 * H], bf16, tag=f"s1_{i}", name=f"s1_{i}")
                   for i in range(2)]
            S2t = [sigpool.tile([H, 4 * H], bf16, tag=f"s2_{i}", name=f"s2_{i}")
                   for i in range(2)]
            TM1 = ewpool.tile([H, TLOC], bf16, tag="TM1")
            T1 = ewpool.tile([H, TLOC], bf16, tag="T1")
            TM2 = ewpool.tile([H, TLOC], bf16, tag="TM2")
            T2 = ewpool.tile([H, TLOC], bf16, tag="T2")
            z1t = [zpool.tile([H, 4 * H], f32, tag=f"z1_{i}", name=f"z1_{i}")
                   for i in range(3)]
            z2t = [zpool.tile([H, 4 * H], f32, tag=f"z2_{i}", name=f"z2_{i}")
                   for i in range(3)]
            fc_ps = fcpool.tile([NCLS, TLOC], f32, tag="fc")

            def z1_wih(b):
                # input-spike AND recurrent-spike matmuls for step b — both
                # have >= 1 iteration of slack, so they are emitted as a
                # prefetch one iteration ahead, leaving only the 3 Whh_h
                # (i,f,g) matmuls between h landing and the sigmoid.
                ch, off = b // SPC, (b % SPC) * TLOC
                xs = spk_t[ch][:, off:off + TLOC]
                Z1 = z1t[b % 3]
                # 64-col granules (27ns each): slack matmuls can block a
                # later-arriving chain matmul on the in-order PE for at most
                # one granule. start=True only on the bank's FIRST granule
                # (start clears the whole bank's has_written bits).
                for q in range(4):
                    qs = slice(q * H, (q + 1) * H)
                    for g in range(4):
                        cs = slice(q * H + g * 32, q * H + (g + 1) * 32)
                        nc.tensor.matmul(Z1[:, cs], w1ih[:, qs],
                                         xs[:, g * 32:(g + 1) * 32],
                                         start=(q == 0 and g == 0), stop=False,
                                         skip_group_check=True)
                sp = spk1[(b - 2) % 3]
                for q in range(4):
                    qs = slice(q * H, (q + 1) * H)
                    for g in range(4):
                        cs = slice(q * H + g * 32, q * H + (g + 1) * 32)
                        nc.tensor.matmul(Z1[:, cs], w1hs[:, qs],
                                         sp[:, g * 32:(g + 1) * 32],
                                         start=False, stop=False,
                                         skip_group_check=True)

            def z2_pre(b):
                # ALL of layer-2 step b's matmuls. Matmuls into the same
                # PSUM tile execute in emission order, so order them by
                # operand arrival: bias (const), spk2(b-2), spk1(b) (this
                # iteration's spike), h2(b-1) (this iteration's tail) last.
                Z2 = z2t[b % 3]
                # bias as four rank-1 matmuls instead of one 512-wide one:
                # same PE time, but the greedy scheduler can no longer park
                # a 213ns slack matmul right in front of a chain matmul —
                # 53ns granules let later-arriving chain work slot between
                for q in range(4):
                    qs = slice(q * H, (q + 1) * H)
                    for g in range(2):
                        cs = slice(q * H + g * 64, q * H + (g + 1) * 64)
                        nc.tensor.matmul(Z2[:, cs], b2l[0:1, qs],
                                         ind[0:1, g * 64:(g + 1) * 64],
                                         start=(q == 0 and g == 0), stop=False,
                                         skip_group_check=True)
                sp2 = spk2[(b - 2) % 3]
                for q in range(4):
                    qs = slice(q * H, (q + 1) * H)
                    for g in range(4):
                        cs = slice(q * H + g * 32, q * H + (g + 1) * 32)
                        nc.tensor.matmul(Z2[:, cs], w2hs[:, qs],
                                         sp2[:, g * 32:(g + 1) * 32],
                                         start=False, stop=False,
                                         skip_group_check=True)
                sp1 = spk1[b % 3]
                for q in range(4):
                    qs = slice(q * H, (q + 1) * H)
                    for g in range(4):
                        cs = slice(q * H + g * 32, q * H + (g + 1) * 32)
                        nc.tensor.matmul(Z2[:, cs], w2ih[:, qs],
                                         sp1[:, g * 32:(g + 1) * 32],
                                         start=False, stop=False,
                                         skip_group_check=True)
                for q in range(3):
                    qs = slice(q * H, (q + 1) * H)
                    nc.tensor.matmul(Z2[:, qs], w2hh[:, qs],
                                     H2t[(b - 1) % 2][:],
                                     start=False, stop=False,
                                     skip_group_check=True)
                # (o-gate Whh_h deferred to the consuming iteration so
                # sig2_ifg doesn't wait for it)

            # Software-pipelined emission: iteration `it` emits layer 1 of
            # step `it` interleaved with layer 2 of step `it-1`, so the two
            # recurrence chains overlap on the in-order engine queues.
            for it in range((B + 2) * reps):
                b1 = it          # layer-1 step
                b2 = it - 2      # layer-2 step (2-iteration lag: every layer-2
                #                 dependency then has >= 1 full iteration of
                #                 slack, so its ACT/DVE ops never push back the
                #                 layer-1 recurrence chain)
                if reps > 1:     # timing mode: keep pipeline structure per rep
                    b1 = it % (B + 2)
                    b2 = b1 - 2
                S1 = S1t[b1 % 2]
                S2 = S2t[b2 % 2]
                # --- PE, in order of dependency arrival. Everything except
                # Z1's Whh_h matmuls (dep: h1, the latest-landing tensor) was
                # prefetched or has >= 1 iteration of slack, so the in-order
                # PE queue clears Whh_h immediately when h1 lands and sig1
                # starts asap.
                if b1 == 0:
                    z1_wih(0)   # not prefetched (no previous iteration)
                if b1 < B:
                    Z1 = z1t[b1 % 3]
                    # Whh_h for i,f,g only — the o-gate matmul is deferred
                    # below so sig1_ifg's frozen wait lands on the g-gate
                    # matmul (53ns earlier); sig1_o has ~600ns of slack.
                    for q in range(3):
                        qs = slice(q * H, (q + 1) * H)
                        nc.tensor.matmul(Z1[:, qs], w1hh[:, qs],
                                         H1t[(b1 - 1) % 2][:],
                                         start=False, stop=False,
                                         skip_group_check=True)
                # --- ACT: i,f,g sigmoid gates the front; o-gates separate
                # (only needed by the tails, ~1.3us later).
                if b1 < B:
                    nc.scalar.activation(S1[:, 0:3 * H], z1t[b1 % 3][:, 0:3 * H],
                                         ACT.Sigmoid)
                if b2 >= 0:
                    nc.scalar.activation(S2[:, 0:3 * H], z2t[b2 % 3][:, 0:3 * H],
                                         ACT.Sigmoid)
                # deferred o-gate recurrent matmuls (carry each bank's
                # stop flag; only the slack o-sigmoids wait on them)
                if b1 < B:
                    qo = slice(3 * H, 4 * H)
                    nc.tensor.matmul(z1t[b1 % 3][:, qo], w1hh[:, qo],
                                     H1t[(b1 - 1) % 2][:], start=False,
                                     stop=True, skip_group_check=True)
                if b2 >= 0:
                    qo = slice(3 * H, 4 * H)
                    nc.tensor.matmul(z2t[b2 % 3][:, qo], w2hh[:, qo],
                                     H2t[(b2 - 1) % 2][:], start=False,
                                     stop=True, skip_group_check=True)
                if b1 < B:
                    nc.scalar.activation(S1[:, 3 * H:4 * H],
                                         z1t[b1 % 3][:, 3 * H:4 * H], ACT.Sigmoid)
                # --- DVE front L1: hc' = sig(f)*hc + (sig(2 z_g)-0.5)*sig(i)
                # T (cheap TT) first, TM (slower TSP) second: hc starts at
                # TM_end + ack either way, so finishing T early wins ~30ns
                if b1 < B:
                    nc.vector.tensor_mul(T1[:], S1[:, H:2 * H], hc1)
                    nc.vector.scalar_tensor_tensor(TM1[:], S1[:, 2 * H:3 * H], 0.5,
                                                   S1[:, 0:H], OP.subtract, OP.mult)
                    nc.vector.tensor_add(hc1, TM1[:], T1[:])
                if b2 >= 0:
                    nc.scalar.activation(S2[:, 3 * H:4 * H],
                                         z2t[b2 % 3][:, 3 * H:4 * H], ACT.Sigmoid)
                if b1 < B:
                    nc.scalar.activation(THC[:, 0:TLOC], hc1, ACT.Tanh, scale=2.0)
                if b2 >= 0:
                    nc.vector.scalar_tensor_tensor(TM2[:], S2[:, 2 * H:3 * H], 0.5,
                                                   S2[:, 0:H], OP.subtract, OP.mult)
                    nc.vector.tensor_mul(T2[:], S2[:, H:2 * H], hc2)
                    nc.vector.tensor_add(hc2, TM2[:], T2[:])
                # --- DVE tail L1: h = sig(o)*tanh — feeds next step's PE
                if b1 < B:
                    nc.vector.tensor_mul(H1t[b1 % 2][:], S1[:, 3 * H:4 * H],
                                         THC[:, 0:TLOC])
                if b2 >= 0:
                    nc.scalar.activation(THC[:, TLOC:2 * TLOC], hc2, ACT.Tanh,
                                         scale=2.0)
                    nc.vector.tensor_mul(H2t[b2 % 2][:], S2[:, 3 * H:4 * H],
                                         THC[:, TLOC:2 * TLOC])
                # prefetch next step's layer-2 bank right after the h2
                # write: highest priority among the slack PE work so its
                # chain-critical Whh_h granules beat the w1hs granules to
                # the PE when h2 lands
                if 0 <= b2 + 1 < B:
                    z2_pre(b2 + 1)
                # spike + running threshold (DVE — the Pool engine only
                # supports integer ALU ops): spk = (h > RP), RP' = thr*spk
                # + thr. Consumed next iteration -> emitted last (lowest
                # scheduler priority), off the critical path.
                if b1 < B:
                    nc.vector.tensor_tensor(spk1[b1 % 3][:], H1t[b1 % 2][:],
                                            RP1[:], OP.is_gt)
                    nc.vector.tensor_scalar(RP1[:], spk1[b1 % 3][:],
                                            thr1, thr1, OP.mult, OP.add)
                if b2 >= 0:
                    nc.vector.tensor_tensor(spk2[b2 % 3][:], H2t[b2 % 2][:],
                                            RP2[:], OP.is_gt)
                    nc.vector.tensor_scalar(RP2[:], spk2[b2 % 3][:],
                                            thr2, thr2, OP.mult, OP.add)
                # remaining slack PE work, lowest priority: next step's
                # layer-1 input matmuls and the fc spike-half
                if b1 + 1 < B:
                    z1_wih(b1 + 1)
                # fc accumulation last (lowest PE priority — its result is
                # only needed at the very end). Reads this iteration's h2
                # and last iteration's spk2 (program order defines both).
                if b2 >= 0:
                    nc.tensor.matmul(fc_ps[:], fcwh[:], H2t[b2 % 2][:],
                                     start=(b2 == 0), stop=False,
                                     skip_group_check=True)
                if b2 - 1 >= 0:
                    # the last-emitted fc op overall carries the group stop
                    nc.tensor.matmul(fc_ps[:], fcws[:], spk2[(b2 - 1) % 3][:],
                                     start=False, stop=(b2 == B - 1),
                                     skip_group_check=True)

            # ---------------- scale + bias + store ----------------
            out_sb = ewpool.tile([NCLS, TLOC], f32, tag="outsb")
            nc.vector.tensor_scalar(out_sb[:], fc_ps[:], 1.0 / B, fcb[:, 0:1],
                                    OP.mult, OP.add)
            nc.sync.dma_start(out_d[:], out_sb[:])

    return nc


def _prep_weights(Wih1, Whh1, bih1, bhh1, Wih2, Whh2, bih2, bhh2, fcW, fcb,
                  thr1, thr2):
    w1ih = np.zeros((KIN, 4 * H), np.float32)
    w1hh = np.zeros((H, 4 * H), np.float32)
    w2ih = np.zeros((H, 4 * H), np.float32)
    w2hh = np.zeros((H, 4 * H), np.float32)
    b2l = np.zeros((1, 4 * H), np.float32)
    ind = np.ones((1, 4 * H), np.float32)
    for qn, og in enumerate(QORDER):
        sc = 2.0 if qn == 2 else 1.0   # tanh-via-sigmoid: z_g pre-scaled by 2
        sl = slice(og * H, (og + 1) * H)
        dn = slice(qn * H, (qn + 1) * H)
        w1ih[0:F3, dn] = sc * Wih1[sl, :].T
        w1ih[F3, dn] = sc * (bih1[sl] + bhh1[sl])
        w1hh[:, dn] = sc * Whh1[sl, :].T       # rhs is h (mem = h - thr*spk)
        w2ih[:, dn] = sc * Wih2[sl, :].T
        w2hh[:, dn] = sc * Whh2[sl, :].T       # rhs is h
        b2l[0, dn] = sc * (bih2[sl] + bhh2[sl])
    w1hs = -thr1 * w1hh                        # rhs is spk_prev (0/1)
    w2hs = -thr2 * w2hh
    fcwh = fcW.T.astype(np.float32)            # rhs is h2
    fcws = -thr2 * fcwh                        # rhs is spk2_prev
    return (w1ih.astype(BF16), w1hh.astype(BF16), w1hs.astype(BF16),
            w2ih.astype(BF16), w2hh.astype(BF16), w2hs.astype(BF16),
            b2l.astype(BF16), ind.astype(BF16),
            fcwh.astype(BF16).copy(), fcws.astype(BF16).copy(),
            fcb.reshape(NCLS, 1).astype(np.float32).copy())


def _spike_encode(x):
    """[B, T, 14] f32 -> [B, T, 42] f32 spikes (exact 0/1)."""
    diff = x[:, 1:] - x[:, :-1]                       # [B, T-1, 14]
    spikes = (diff[..., None] > THRESHOLDS).astype(np.float32)
    sd = np.zeros((x.shape[0], x.shape[1], F3), np.float32)
    sd[:, 1:] = spikes.reshape(x.shape[0], x.shape[1] - 1, F3)
    return sd


def kernel(**inputs):
    global LAST_RESULT
    x = np.asarray(inputs["x"], np.float32)
    thr1 = float(np.asarray(inputs["thr1"]))
    thr2 = float(np.asarray(inputs["thr2"]))
    (w1ih, w1hh, w1hs, w2ih, w2hh, w2hs, b2l, ind, fcwh, fcws, fcb) = \
        _prep_weights(
            np.asarray(inputs["Wih1"], np.float32),
            np.asarray(inputs["Whh1"], np.float32),
            np.asarray(inputs["bih1"], np.float32),
            np.asarray(inputs["bhh1"], np.float32),
            np.asarray(inputs["Wih2"], np.float32),
            np.asarray(inputs["Whh2"], np.float32),
            np.asarray(inputs["bih2"], np.float32),
            np.asarray(inputs["bhh2"], np.float32),
            np.asarray(inputs["fcW"], np.float32),
            np.asarray(inputs["fcb"], np.float32), thr1, thr2)

    sd = _spike_encode(x)  # [B, T, 42]
    shared = dict(w1ih=w1ih, w1hh=w1hh, w1hs=w1hs, w2ih=w2ih, w2hh=w2hh,
                  w2hs=w2hs, b2l=b2l, ind=ind, fcwh=fcwh, fcws=fcws, fcb=fcb)
    in_maps = []
    for d in range(NCORES):
        sl = sd[:, d * TLOC:(d + 1) * TLOC, :]            # [B, TLOC, 42]
        sp = np.ascontiguousarray(np.transpose(sl, (2, 0, 1))).reshape(F3, B * TLOC)
        spk = np.concatenate([sp, np.ones((1, B * TLOC), np.float32)], 0).astype(BF16)
        in_maps.append(dict(spk=spk, **shared))

    reps = int(os.environ.get("KERNEL_REPS", "1"))
    nc = _build(thr1, thr2, reps)
    nc.finalize()  # Bacc: runs wait-splitting + reg alloc before serialization
    trace = os.environ.get("KERNEL_TRACE", "0") == "1"
    try:
        res = run_bass_kernel_spmd(nc, in_maps, core_ids=list(range(NCORES)),
                                   trace=trace)
    except ModuleNotFoundError:
        res = run_bass_kernel_spmd(nc, in_maps, core_ids=list(range(NCORES)),
                                   trace=False)
    LAST_RESULT = res
    out = np.concatenate([r["out"].T for r in res.results], axis=0)  # [1024, 8]
    return np.ascontiguousarray(out.astype(np.float32))
